# revision 13
# baseline (speedup 1.0000x reference)
"""GCN classifier (2x 4-layer GraphConv branches + segment-mean readout)
on 8 TRN2 NeuronCores.

Strategy:
  - Nodes partitioned across 8 cores by graph (16 graphs/core, contiguous
    node ranges since seg is sorted). Edges assigned to the core owning dst.
  - Hidden state lives in a packed DRAM table [8*NB*128, 128] (AllGather of
    per-core shards). Layer-k+1 per-edge features are fetched with
    dma_gather (int16 idx -> 4 sub-table views of <=32768 rows).
  - Scatter-add (segment_sum over dst) = one-hot matmul on the tensor
    engine accumulating in PSUM per 128-node block.
  - Per-node D^-1/2 norms folded into PSUM evictions (per-partition
    tensor_scalar), dense W matmul in transposed layout, bias+ReLU on ACT.
  - Layer 1 (features = in-degree, rank-1) is collapsed on host into a
    per-node scalar a = cd * segsum((in_deg*cs)[src]); h1 = relu(outer(a,
    W1) + b1) built on device with 2 DVE ops per block.
  - Readout = matmul with per-graph one-hot mask accumulated over blocks.
  - Final |hg1-hg2| @ Wc + bc is a [128,128]x[128,10] op done on host.
"""

import os
import numpy as np

N_NODES = 100000
N_EDGES = 1600000
N_GRAPHS = 128
HIDDEN = 128
N_CLASSES = 10
NCORES = 8
GPC = N_GRAPHS // NCORES  # graphs per core
BLK = 128
GBLK = 4  # blocks per gather group
BUCKET = 32768

LAST_RESULTS = None  # BassKernelResults of the most recent hardware run
_NC_CACHE = {}


# ----------------------------------------------------------------- host side

def _wrap_idx(seq):
    """[L] -> [128, L//16] int16: idx j at partition j%16, col j//16,
    replicated across the 8 groups of 16 partitions."""
    L = seq.shape[0]
    assert L % 16 == 0
    w = seq.reshape(L // 16, 16).T.astype(np.int16)
    return np.tile(w, (8, 1))


def _preprocess_branch(src, dst, seg):
    """Per-branch host preprocessing. Returns dict with per-core packed
    arrays (before device layout) + per-core meta."""
    src = np.asarray(src).astype(np.int64)
    dst = np.asarray(dst).astype(np.int64)
    seg = np.asarray(seg).astype(np.int64)

    ones = np.ones(N_EDGES, np.float64)
    out_deg = np.bincount(src, minlength=N_NODES).astype(np.float32)
    in_deg = np.bincount(dst, minlength=N_NODES).astype(np.float32)
    cs = (1.0 / np.sqrt(np.maximum(out_deg, 1.0))).astype(np.float32)
    cd = (1.0 / np.sqrt(np.maximum(in_deg, 1.0))).astype(np.float32)
    agg1 = np.bincount(dst, weights=(in_deg * cs)[src].astype(np.float64),
                       minlength=N_NODES).astype(np.float32)
    a = agg1 * cd

    bounds = np.searchsorted(seg, np.arange(0, N_GRAPHS + 1, GPC))
    sizes = np.diff(bounds)  # nodes per core
    gcounts = np.bincount(seg, minlength=N_GRAPHS).astype(np.float32)

    cores = []
    for c in range(NCORES):
        n0, n1 = int(bounds[c]), int(bounds[c + 1])
        m = (dst >= n0) & (dst < n1)
        e_src = src[m]
        e_dst = dst[m]
        cores.append(dict(n0=n0, n1=n1, e_src=e_src, e_dst=e_dst))
    return dict(cs=cs, cd=cd, a=a, bounds=bounds, sizes=sizes,
                gcounts=gcounts, cores=cores, seg=seg)


def _finish_packing(pre, NB, CB):
    """Given common NB (blocks/core) and CB (chunk caps per bucket),
    build per-core device arrays for one branch."""
    NCH = int(np.sum(CB))
    NGRP = (NB + GBLK - 1) // GBLK
    NBp = NGRP * GBLK
    R = NCORES * NBp * BLK
    cbase = np.concatenate([[0], np.cumsum(CB)]).astype(np.int64)

    bounds = pre["bounds"]
    cs, cd, a, seg = pre["cs"], pre["cd"], pre["a"], pre["seg"]

    out = []
    for c in range(NCORES):
        cc = pre["cores"][c]
        n0, n1 = cc["n0"], cc["n1"]
        size = n1 - n0
        e_src, e_dst = cc["e_src"], cc["e_dst"]

        # packed (remapped) source rows
        src_core = np.searchsorted(bounds, e_src, side="right") - 1
        p_row = src_core * (NBp * BLK) + (e_src - bounds[src_core])
        beta = p_row >> 15
        idx16 = p_row & (BUCKET - 1)

        blk = (e_dst - n0) >> 7
        dstloc = (e_dst - n0) & 127

        key = blk * 4 + beta
        order = np.argsort(key, kind="stable")
        key_s = key[order]
        cnts = np.bincount(key_s, minlength=NB * 4)
        starts = np.concatenate([[0], np.cumsum(cnts)[:-1]])
        pos = np.arange(len(key_s)) - np.repeat(starts, cnts)
        blk_s, beta_s = blk[order], beta[order]
        chunk_i = pos >> 7
        part = pos & 127
        assert (chunk_i < CB[beta_s]).all(), "bucket cap overflow"

        col = blk_s * NCH + cbase[beta_s] + chunk_i
        dst_full = np.full((NBp * NCH, BLK), -1.0, np.float32)
        idx_full = np.zeros((NBp * NCH, BLK), np.int64)
        dst_full[col, part] = dstloc[order].astype(np.float32)
        idx_full[col, part] = idx16[order]

        # device idx stream: per group, per bucket, blocks-minor
        idx_dev_cols = []
        for g in range(NGRP):
            for b in range(4):
                cols = []
                for bg in range(GBLK):
                    bb = g * GBLK + bg
                    cols.extend(bb * NCH + cbase[b] + i for i in range(CB[b]))
                seq = idx_full[cols].reshape(-1)  # [G*CB[b]*128]
                if len(seq):
                    idx_dev_cols.append(_wrap_idx(seq))
        idx_dev = np.concatenate(idx_dev_cols, axis=1)  # [128, NGRP*NCH*G*8]
        dst_dev = dst_full.T.copy()  # [128, NBp*NCH]

        def pack_nodes(vals, pad):
            pk = np.full(NBp * BLK, pad, np.float32)
            pk[:size] = vals[n0:n1]
            return pk.reshape(NBp, BLK).T.copy()  # [128, NBp]

        apk = pack_nodes(a, 0.0)
        cspk = pack_nodes(cs, 0.0)
        cdpk = pack_nodes(cd, 0.0)
        segpk = pack_nodes((seg - c * GPC).astype(np.float32), -1.0)
        invc = (1.0 / np.maximum(pre["gcounts"][c * GPC:(c + 1) * GPC], 1.0))
        invc = invc.astype(np.float32)[:, None]

        out.append(dict(idx=idx_dev, dst=dst_dev, apk=apk, cs=cspk, cd=cdpk,
                        seg=segpk, invc=invc))
    return out, dict(NB=NB, CB=tuple(int(x) for x in CB), NCH=NCH,
                     NGRP=NGRP, NBp=NBp, R=R)


def _branch_caps(pre, NB):
    """Max chunks per (block, bucket) over cores for one branch."""
    NBp = ((NB + GBLK - 1) // GBLK) * GBLK
    CB = np.zeros(4, np.int64)
    for c in range(NCORES):
        cc = pre["cores"][c]
        n0 = cc["n0"]
        e_src, e_dst = cc["e_src"], cc["e_dst"]
        src_core = np.searchsorted(pre["bounds"], e_src, side="right") - 1
        p_row = src_core * (NBp * BLK) + (e_src - pre["bounds"][src_core])
        beta = p_row >> 15
        blk = (e_dst - n0) >> 7
        cnt = np.bincount(blk * 4 + beta, minlength=NB * 4).reshape(NB, 4)
        CB = np.maximum(CB, (cnt + 127) // 128, casting="unsafe").astype(np.int64) \
            if False else np.maximum(CB, ((cnt + 127) // 128).max(axis=0))
    return CB


# --------------------------------------------------------- numpy device sim

def _run_numpy_sim(devs, meta, W):
    """Full 8-core simulation of one branch with AG between layers."""
    NBp = meta["NBp"]
    h1 = []
    W1r, b1r = W[0], W[1]
    for c in range(NCORES):
        shard = np.zeros((NBp * BLK, HIDDEN), np.float32)
        for blk in range(NBp):
            a_col = devs[c]["apk"][:, blk][:, None]
            cs_col = devs[c]["cs"][:, blk][:, None]
            t = W1r * a_col + b1r
            shard[blk * BLK:(blk + 1) * BLK] = np.maximum(t, 0.0) * cs_col
        h1.append(shard)
    tables = [np.concatenate(h1, axis=0)]
    hgs = [None] * NCORES
    cur = h1
    for li in range(3):
        table = np.concatenate(cur, axis=0)
        nxt = []
        for c in range(NCORES):
            dev = devs[c]
            s, hg = _sim_core_layer(dev, meta, W, li, table)
            if li < 2:
                nxt.append(s)
            else:
                hgs[c] = hg
        cur = nxt
    return np.concatenate(hgs, axis=0)


def _sim_core_layer(dev, meta, W, li, table):
    NB, NCH, NGRP, CB = meta["NB"], meta["NCH"], meta["NGRP"], meta["CB"]
    NBp = meta["NBp"]
    cbase = np.concatenate([[0], np.cumsum(CB)]).astype(np.int64)
    Wl = W[2 + 2 * li]
    bl = W[3 + 2 * li]
    Rr = table.shape[0]
    iota = np.arange(128, dtype=np.float32)
    newshard = np.zeros((NBp * BLK, HIDDEN), np.float32)
    psum_hg = np.zeros((16, HIDDEN), np.float32)
    colptr = 0
    idx_dev = dev["idx"]
    for g in range(NGRP):
        msg = np.zeros((128, GBLK * NCH, HIDDEN), np.float32)
        slot = 0
        for bkt in range(4):
            ncol8 = GBLK * CB[bkt] * 8
            if ncol8 == 0:
                continue
            wrapped = idx_dev[:16, colptr:colptr + ncol8]
            seq = wrapped.T.reshape(-1).astype(np.int64)
            colptr += ncol8
            view = table[bkt * BUCKET:min((bkt + 1) * BUCKET, Rr)]
            gath = view[seq]
            nslots = GBLK * CB[bkt]
            msg[:, slot:slot + nslots, :] = \
                gath.reshape(nslots, 128, HIDDEN).transpose(1, 0, 2)
            slot += nslots
        for bg in range(GBLK):
            bb = g * GBLK + bg
            agg = np.zeros((128, HIDDEN), np.float32)
            for bkt in range(4):
                for i in range(CB[bkt]):
                    s = cbase[bkt] * GBLK + bg * CB[bkt] + i
                    colx = bb * NCH + cbase[bkt] + i
                    d = dev["dst"][:, colx]
                    oh = (iota[None, :] == d[:, None]).astype(np.float32)
                    agg += oh.T @ msg[:, s, :]
            aggs = agg * dev["cd"][:, bb][:, None]
            tmpT = Wl.T @ aggs.T
            h = np.maximum(tmpT + bl[:, None], 0.0).T  # [n, f]
            if li < 2:
                newshard[bb * BLK:(bb + 1) * BLK] = \
                    h * dev["cs"][:, bb][:, None]
            else:
                segc = dev["seg"][:, bb]
                mask = (iota[None, :16] == segc[:, None]).astype(np.float32)
                psum_hg += mask.T @ h
    if li < 2:
        return newshard, None
    return None, psum_hg * dev["invc"]


# ------------------------------------------------------------- bass builder

def _build_nc(meta):
    import concourse.bacc as bacc
    import concourse.mybir as mybir
    import concourse.tile as tile

    dt = mybir.dt
    NB, CB, NCH, NGRP, NBp, R = (meta["NB"], meta["CB"], meta["NCH"],
                                 meta["NGRP"], meta["NBp"], meta["R"])
    cbase = np.concatenate([[0], np.cumsum(CB)]).astype(np.int64)
    IDXC = NCH * GBLK * 8  # idx cols per group
    nviews = [(b * BUCKET, min((b + 1) * BUCKET, R)) for b in range(4)
              if b * BUCKET < R]

    stages = os.environ.get("KBUILD_STAGES", "full")
    nlayers = {"h1": 0, "l2": 1, "l3": 2, "full": 3}[stages]
    grp_lim = int(os.environ.get("KBUILD_NGRP", "0")) or None
    nogather = bool(int(os.environ.get("KBUILD_NOGATHER", "0")))
    localtab = bool(int(os.environ.get("KBUILD_LOCALTAB", "0")))
    nc = bacc.Bacc("TRN2", target_bir_lowering=False, debug=False,
                   enable_asserts=False, num_devices=NCORES)

    ins = {}
    for br in range(2):
        ins[f"idx{br}"] = nc.dram_tensor(f"idx{br}", [128, NGRP * IDXC],
                                         dt.int16, kind="ExternalInput")
        ins[f"dst{br}"] = nc.dram_tensor(f"dst{br}", [128, NBp * NCH],
                                         dt.float32, kind="ExternalInput")
        for nm in ("apk", "cs", "cd", "seg"):
            ins[f"{nm}{br}"] = nc.dram_tensor(f"{nm}{br}", [128, NBp],
                                              dt.float32, kind="ExternalInput")
        ins[f"invc{br}"] = nc.dram_tensor(f"invc{br}", [16, 1], dt.float32,
                                          kind="ExternalInput")
    for nm in ("W1r", "b1r", "W2", "W3", "W4", "iota", "ident"):
        ins[nm] = nc.dram_tensor(nm, [128, 128], dt.float32,
                                 kind="ExternalInput")
    for nm in ("b2c", "b3c", "b4c"):
        ins[nm] = nc.dram_tensor(nm, [128, 1], dt.float32,
                                 kind="ExternalInput")
    outs = [nc.dram_tensor(f"hg{br}", [16, 128], dt.float32,
                           kind="ExternalOutput") for br in range(2)]

    with tile.TileContext(nc) as tc:
        with tc.tile_pool(name="dram", bufs=1, space="DRAM") as dram, \
             tc.tile_pool(name="const", bufs=1) as cpool, \
             tc.tile_pool(name="nodes", bufs=1) as npool, \
             tc.tile_pool(name="stream", bufs=3) as spool, \
             tc.tile_pool(name="msgp", bufs=2) as mpool, \
             tc.tile_pool(name="ohp", bufs=4) as ohpool, \
             tc.tile_pool(name="workp", bufs=3) as wpool, \
             tc.tile_pool(name="psA", bufs=2, space="PSUM") as psA, \
             tc.tile_pool(name="psB", bufs=2, space="PSUM") as psB, \
             tc.tile_pool(name="psC", bufs=2, space="PSUM") as psC, \
             tc.tile_pool(name="psD", bufs=1, space="PSUM") as psD, \
             tc.tile_pool(name="psE", bufs=1, space="PSUM") as psE:

            shard = dram.tile([NBp * BLK, 128], dt.float32)
            tables_br = [[dram.tile([R, 128], dt.float32,
                                    name=f"table{br}_{i}",
                                    tag=f"table{br}_{i}",
                                    addr_space="Shared")
                          for i in range(3)] for br in range(2)]

            consts = {}
            for nm in ("W1r", "b1r", "W2", "W3", "W4", "iota", "ident"):
                t = cpool.tile([128, 128], dt.float32, tag=nm)
                nc.sync.dma_start(t[:], ins[nm][:])
                consts[nm] = t
            for nm in ("b2c", "b3c", "b4c"):
                t = cpool.tile([128, 1], dt.float32, tag=nm)
                nc.sync.dma_start(t[:], ins[nm][:])
                consts[nm] = t

            for br in range(2):
                tables = tables_br[br]
                # per-branch node arrays resident in SBUF
                nodes = {}
                for nm in ("apk", "cs", "cd", "seg"):
                    t = npool.tile([128, NBp], dt.float32, tag=nm)
                    nc.sync.dma_start(t[:], ins[f"{nm}{br}"][:])
                    nodes[nm] = t
                invc = npool.tile([16, 1], dt.float32, tag="invc")
                nc.sync.dma_start(invc[:], ins[f"invc{br}"][:])

                # ---- h1 phase
                for blk in range(NBp):
                    t1 = wpool.tile([128, 128], dt.float32, tag="h1t")
                    nc.vector.scalar_tensor_tensor(
                        t1[:], consts["W1r"][:], nodes["apk"][:, blk:blk + 1],
                        consts["b1r"][:],
                        mybir.AluOpType.mult, mybir.AluOpType.add)
                    h1s = wpool.tile([128, 128], dt.float32, tag="h1s")
                    nc.vector.tensor_scalar(
                        h1s[:], t1[:], 0.0, nodes["cs"][:, blk:blk + 1],
                        mybir.AluOpType.max, mybir.AluOpType.mult)
                    nc.sync.dma_start(shard[blk * BLK:(blk + 1) * BLK, :],
                                      h1s[:])
                nc.gpsimd.collective_compute(
                    "AllGather", mybir.AluOpType.bypass,
                    replica_groups=[list(range(NCORES))],
                    ins=[shard.opt()], outs=[tables[0].opt()])

                # ---- layers 2..4
                for li in range(nlayers):
                    table = tables[li]
                    if localtab:
                        tl = dram.tile([R, 128], dt.float32, name=f"tl{br}_{li}",
                                       tag=f"tloc{li}")
                        nc.sync.dma_start(tl[:], table[:])
                        table = tl
                    Wl = consts[("W2", "W3", "W4")[li]]
                    blc = consts[("b2c", "b3c", "b4c")[li]]
                    last = (li == 2)
                    if stages != "full" and li == nlayers - 1:
                        last = False  # skip readout when truncated
                    if last:
                        hg_ps = psE.tile([16, 128], dt.float32, tag="hg")
                    for g in range(NGRP if grp_lim is None else min(grp_lim, NGRP)):
                        idxt = spool.tile([128, IDXC], dt.int16, tag="idxt")
                        nc.sync.dma_start(
                            idxt[:], ins[f"idx{br}"][:, g * IDXC:(g + 1) * IDXC])
                        dstt = spool.tile([128, GBLK * NCH], dt.float32,
                                          tag="dstt")
                        nc.sync.dma_start(
                            dstt[:],
                            ins[f"dst{br}"][:, g * GBLK * NCH:(g + 1) * GBLK * NCH])
                        msg = mpool.tile([128, GBLK * NCH, 128], dt.float32,
                                         tag="msg")
                        ccol = 0
                        for bkt in range(4):
                            L = GBLK * CB[bkt] * 128
                            if L == 0:
                                continue
                            v0, v1 = bkt * BUCKET, min((bkt + 1) * BUCKET, R)
                            s0 = cbase[bkt] * GBLK
                            nslots = GBLK * CB[bkt]
                            if nogather:
                                nc.sync.dma_start(
                                    msg[:, s0:s0 + nslots, :],
                                    table[0:nslots * 128, :].rearrange(
                                        "(p s) f -> p s f", p=128))
                            else:
                                nc.gpsimd.dma_gather(
                                    msg[:, s0:s0 + nslots, :], table[v0:v1, :],
                                    idxt[:, ccol:ccol + L // 16], L, L, 128,
                                    single_packet=False)
                            ccol += L // 16
                        for bg in range(GBLK):
                            bb = g * GBLK + bg
                            agg_ps = psA.tile([128, 128], dt.float32,
                                              tag="agg")
                            nchunk = 0
                            for bkt in range(4):
                                for i in range(CB[bkt]):
                                    s = cbase[bkt] * GBLK + bg * CB[bkt] + i
                                    colx = (bg * NCH + cbase[bkt] + i)
                                    oh = ohpool.tile([128, 128], dt.float32,
                                                     tag="oh")
                                    nc.vector.tensor_scalar(
                                        oh[:], consts["iota"][:],
                                        dstt[:, colx:colx + 1], None,
                                        mybir.AluOpType.is_equal)
                                    nc.tensor.matmul(
                                        agg_ps[:], oh[:], msg[:, s, :],
                                        start=(nchunk == 0),
                                        stop=(nchunk == NCH - 1))
                                    nchunk += 1
                            aggs = wpool.tile([128, 128], dt.float32,
                                              tag="aggs")
                            nc.vector.tensor_scalar(
                                aggs[:], agg_ps[:], nodes["cd"][:, bb:bb + 1],
                                None, mybir.AluOpType.mult)
                            aggT_ps = psB.tile([128, 128], dt.float32,
                                               tag="aggT")
                            nc.tensor.transpose(aggT_ps[:], aggs[:],
                                                consts["ident"][:])
                            aggT = wpool.tile([128, 128], dt.float32,
                                              tag="aggT_s")
                            nc.any.tensor_copy(aggT[:], aggT_ps[:])
                            tmp_ps = psC.tile([128, 128], dt.float32,
                                              tag="tmp")
                            nc.tensor.matmul(tmp_ps[:], Wl[:], aggT[:],
                                             start=True, stop=True)
                            hT = wpool.tile([128, 128], dt.float32, tag="hT")
                            nc.scalar.activation(
                                hT[:], tmp_ps[:],
                                mybir.ActivationFunctionType.Relu,
                                bias=blc[:], scale=1.0)
                            h_ps = psD.tile([128, 128], dt.float32, tag="hf")
                            nc.tensor.transpose(h_ps[:], hT[:],
                                                consts["ident"][:])
                            if not last:
                                hs = wpool.tile([128, 128], dt.float32,
                                                tag="hs")
                                nc.vector.tensor_scalar(
                                    hs[:], h_ps[:], nodes["cs"][:, bb:bb + 1],
                                    None, mybir.AluOpType.mult)
                                nc.sync.dma_start(
                                    shard[bb * BLK:(bb + 1) * BLK, :], hs[:])
                            else:
                                h4 = wpool.tile([128, 128], dt.float32,
                                                tag="hs")
                                nc.any.tensor_copy(h4[:], h_ps[:])
                                mask = ohpool.tile([128, 16], dt.float32,
                                                   tag="mask")
                                nc.vector.tensor_scalar(
                                    mask[:], consts["iota"][:, 0:16],
                                    nodes["seg"][:, bb:bb + 1], None,
                                    mybir.AluOpType.is_equal)
                                nc.tensor.matmul(
                                    hg_ps[:], mask[:], h4[:],
                                    start=(bb == 0), stop=(bb == NBp - 1))
                    if not last:
                        if li + 1 < 3:
                            nc.gpsimd.collective_compute(
                                "AllGather", mybir.AluOpType.bypass,
                                replica_groups=[list(range(NCORES))],
                                ins=[shard.opt()], outs=[tables[li + 1].opt()])
                    else:
                        hg_sb = wpool.tile([16, 128], dt.float32, tag="hgsb")
                        nc.vector.tensor_scalar(
                            hg_sb[:], hg_ps[:], invc[:], None,
                            mybir.AluOpType.mult)
                        nc.sync.dma_start(outs[br][:], hg_sb[:])
    nc.compile()
    return nc


# ------------------------------------------------------------------ driver

def _make_in_maps(devs1, devs2, meta, weights):
    (W1, b1, W2, b2, W3, b3, W4, b4) = weights
    W1r = np.tile(np.asarray(W1, np.float32).reshape(1, 128), (128, 1))
    b1r = np.tile(np.asarray(b1, np.float32).reshape(1, 128), (128, 1))
    iota_np = np.tile(np.arange(128, dtype=np.float32)[None, :], (128, 1))
    ident_np = np.eye(128, dtype=np.float32)
    shared = {
        "W1r": W1r, "b1r": b1r,
        "W2": np.asarray(W2, np.float32), "W3": np.asarray(W3, np.float32),
        "W4": np.asarray(W4, np.float32),
        "b2c": np.asarray(b2, np.float32)[:, None],
        "b3c": np.asarray(b3, np.float32)[:, None],
        "b4c": np.asarray(b4, np.float32)[:, None],
        "iota": iota_np, "ident": ident_np,
    }
    in_maps = []
    for c in range(NCORES):
        m = dict(shared)
        for br, devs in ((0, devs1), (1, devs2)):
            d = devs[c]
            m[f"idx{br}"] = np.ascontiguousarray(d["idx"])
            m[f"dst{br}"] = np.ascontiguousarray(d["dst"])
            m[f"apk{br}"] = np.ascontiguousarray(d["apk"])
            m[f"cs{br}"] = np.ascontiguousarray(d["cs"])
            m[f"cd{br}"] = np.ascontiguousarray(d["cd"])
            m[f"seg{br}"] = np.ascontiguousarray(d["seg"])
            m[f"invc{br}"] = d["invc"]
        in_maps.append(m)
    return in_maps


def _prepare(src1, dst1, seg1, src2, dst2, seg2):
    pre1 = _preprocess_branch(src1, dst1, seg1)
    pre2 = _preprocess_branch(src2, dst2, seg2)
    NB = int(max((np.diff(pre1["bounds"]).max() + BLK - 1) // BLK,
                 (np.diff(pre2["bounds"]).max() + BLK - 1) // BLK))
    CB = np.maximum(_branch_caps(pre1, NB), _branch_caps(pre2, NB))
    devs1, meta = _finish_packing(pre1, NB, CB)
    devs2, meta2 = _finish_packing(pre2, NB, CB)
    assert meta == meta2
    return pre1, pre2, devs1, devs2, meta


def _ensure_ntff_hook():
    """bass_utils imports antenv.axon_hooks for trace=True under axon; this
    image's antenv lacks it. Register an equivalent ctypes-based hook."""
    import sys
    try:
        import antenv.axon_hooks  # noqa: F401
        return
    except ImportError:
        pass
    import contextlib
    import ctypes
    import types

    so_path = os.environ.get("PJRT_LIBRARY_PATH", "/opt/axon/libaxon_pjrt.so")

    def _make_hook():
        try:
            lib = ctypes.CDLL(so_path)
        except OSError:
            return None
        if not hasattr(lib, "axon_start_nrt_profile"):
            return None
        lib.axon_start_nrt_profile.argtypes = [
            ctypes.POINTER(ctypes.c_int64), ctypes.c_size_t]
        lib.axon_start_nrt_profile.restype = ctypes.c_int64
        lib.axon_stop_nrt_profile.argtypes = [ctypes.c_char_p]
        lib.axon_stop_nrt_profile.restype = ctypes.c_int64

        @contextlib.contextmanager
        def _hook(output_dir, device_ids):
            import jax
            jax.devices()
            if device_ids:
                ids = (ctypes.c_int64 * len(device_ids))(*device_ids)
                rc = lib.axon_start_nrt_profile(ids, len(device_ids))
            else:
                rc = lib.axon_start_nrt_profile(None, 0)
            if rc != 0:
                raise RuntimeError(f"axon_start_nrt_profile rc={rc}")
            try:
                yield
            finally:
                n = lib.axon_stop_nrt_profile(str(output_dir).encode())
                if n <= 0:
                    print(f"ntff profile: {n} files written", flush=True)

        return _hook

    mod = types.ModuleType("antenv.axon_hooks")
    _the_hook = _make_hook()
    mod.get_axon_ntff_profile_hook = lambda: _the_hook
    mod.set_axon_ntff_profile_hook = lambda h: None
    sys.modules["antenv.axon_hooks"] = mod


def kernel(src1, dst1, seg1, src2, dst2, seg2,
           W1, b1, W2, b2, W3, b3, W4, b4, Wc, bc):
    global LAST_RESULTS
    from concourse.bass_utils import run_bass_kernel_spmd

    pre1, pre2, devs1, devs2, meta = _prepare(src1, dst1, seg1,
                                              src2, dst2, seg2)
    weights = (W1, b1, W2, b2, W3, b3, W4, b4)
    in_maps = _make_in_maps(devs1, devs2, meta,
                            tuple(np.asarray(w, np.float32) for w in weights))

    key = (meta["NB"], meta["CB"], os.environ.get("KBUILD_STAGES", "full"),
           os.environ.get("KBUILD_NGRP", "0"),
           os.environ.get("KBUILD_NOGATHER", "0"),
           os.environ.get("KBUILD_LOCALTAB", "0"))
    if key not in _NC_CACHE:
        _NC_CACHE[key] = _build_nc(meta)
    nc = _NC_CACHE[key]

    trace = bool(int(os.environ.get("KERNEL_TRACE", "0")))
    if trace:
        _ensure_ntff_hook()
    res = run_bass_kernel_spmd(nc, in_maps, core_ids=list(range(NCORES)),
                               trace=trace)
    LAST_RESULTS = res
    hg1 = np.concatenate([res.results[c]["hg0"] for c in range(NCORES)], 0)
    hg2 = np.concatenate([res.results[c]["hg1"] for c in range(NCORES)], 0)
    hg = np.abs(hg1 - hg2)
    logits = hg @ np.asarray(Wc, np.float32) + np.asarray(bc, np.float32)
    return (hg1.astype(np.float32), hg2.astype(np.float32),
            logits.astype(np.float32))


def kernel_numpy(src1, dst1, seg1, src2, dst2, seg2,
                 W1, b1, W2, b2, W3, b3, W4, b4, Wc, bc):
    """Host-only simulation of the exact device algorithm (for validation)."""
    pre1, pre2, devs1, devs2, meta = _prepare(src1, dst1, seg1,
                                              src2, dst2, seg2)
    W1r = np.tile(np.asarray(W1, np.float32).reshape(1, 128), (128, 1))
    b1r = np.tile(np.asarray(b1, np.float32).reshape(1, 128), (128, 1))
    W = (W1r, b1r, np.asarray(W2, np.float32), np.asarray(b2, np.float32),
         np.asarray(W3, np.float32), np.asarray(b3, np.float32),
         np.asarray(W4, np.float32), np.asarray(b4, np.float32))
    hg1 = _run_numpy_sim(devs1, meta, W)
    hg2 = _run_numpy_sim(devs2, meta, W)
    hg = np.abs(hg1 - hg2)
    logits = hg @ np.asarray(Wc, np.float32) + np.asarray(bc, np.float32)
    return hg1, hg2, logits


# revision 14
# speedup vs baseline: 1.0041x; 1.0041x over previous
"""GCN classifier (2x 4-layer GraphConv branches + segment-mean readout)
on 8 TRN2 NeuronCores.

Strategy:
  - Nodes partitioned across 8 cores by graph (16 graphs/core, contiguous
    node ranges since seg is sorted). Edges assigned to the core owning dst.
  - Hidden state lives in a packed DRAM table [8*NB*128, 128] (AllGather of
    per-core shards). Layer-k+1 per-edge features are fetched with
    dma_gather (int16 idx -> 4 sub-table views of <=32768 rows).
  - Scatter-add (segment_sum over dst) = one-hot matmul on the tensor
    engine accumulating in PSUM per 128-node block.
  - Per-node D^-1/2 norms folded into PSUM evictions (per-partition
    tensor_scalar), dense W matmul in transposed layout, bias+ReLU on ACT.
  - Layer 1 (features = in-degree, rank-1) is collapsed on host into a
    per-node scalar a = cd * segsum((in_deg*cs)[src]); h1 = relu(outer(a,
    W1) + b1) built on device with 2 DVE ops per block.
  - Readout = matmul with per-graph one-hot mask accumulated over blocks.
  - Final |hg1-hg2| @ Wc + bc is a [128,128]x[128,10] op done on host.
"""

import os
import numpy as np

N_NODES = 100000
N_EDGES = 1600000
N_GRAPHS = 128
HIDDEN = 128
N_CLASSES = 10
NCORES = 8
GPC = N_GRAPHS // NCORES  # graphs per core
BLK = 128
GBLK = 4  # blocks per gather group
BUCKET = 32768

LAST_RESULTS = None  # BassKernelResults of the most recent hardware run
_NC_CACHE = {}


# ----------------------------------------------------------------- host side

def _wrap_idx(seq):
    """[L] -> [128, L//16] int16: idx j at partition j%16, col j//16,
    replicated across the 8 groups of 16 partitions."""
    L = seq.shape[0]
    assert L % 16 == 0
    w = seq.reshape(L // 16, 16).T.astype(np.int16)
    return np.tile(w, (8, 1))


def _preprocess_branch(src, dst, seg):
    """Per-branch host preprocessing. Returns dict with per-core packed
    arrays (before device layout) + per-core meta."""
    src = np.asarray(src).astype(np.int64)
    dst = np.asarray(dst).astype(np.int64)
    seg = np.asarray(seg).astype(np.int64)

    ones = np.ones(N_EDGES, np.float64)
    out_deg = np.bincount(src, minlength=N_NODES).astype(np.float32)
    in_deg = np.bincount(dst, minlength=N_NODES).astype(np.float32)
    cs = (1.0 / np.sqrt(np.maximum(out_deg, 1.0))).astype(np.float32)
    cd = (1.0 / np.sqrt(np.maximum(in_deg, 1.0))).astype(np.float32)
    agg1 = np.bincount(dst, weights=(in_deg * cs)[src].astype(np.float64),
                       minlength=N_NODES).astype(np.float32)
    a = agg1 * cd

    bounds = np.searchsorted(seg, np.arange(0, N_GRAPHS + 1, GPC))
    sizes = np.diff(bounds)  # nodes per core
    gcounts = np.bincount(seg, minlength=N_GRAPHS).astype(np.float32)

    cores = []
    for c in range(NCORES):
        n0, n1 = int(bounds[c]), int(bounds[c + 1])
        m = (dst >= n0) & (dst < n1)
        e_src = src[m]
        e_dst = dst[m]
        cores.append(dict(n0=n0, n1=n1, e_src=e_src, e_dst=e_dst))
    return dict(cs=cs, cd=cd, a=a, bounds=bounds, sizes=sizes,
                gcounts=gcounts, cores=cores, seg=seg)


def _finish_packing(pre, NB, CB):
    """Given common NB (blocks/core) and CB (chunk caps per bucket),
    build per-core device arrays for one branch."""
    NCH = int(np.sum(CB))
    NGRP = (NB + GBLK - 1) // GBLK
    NBp = NGRP * GBLK
    R = NCORES * NBp * BLK
    cbase = np.concatenate([[0], np.cumsum(CB)]).astype(np.int64)

    bounds = pre["bounds"]
    cs, cd, a, seg = pre["cs"], pre["cd"], pre["a"], pre["seg"]

    out = []
    for c in range(NCORES):
        cc = pre["cores"][c]
        n0, n1 = cc["n0"], cc["n1"]
        size = n1 - n0
        e_src, e_dst = cc["e_src"], cc["e_dst"]

        # packed (remapped) source rows
        src_core = np.searchsorted(bounds, e_src, side="right") - 1
        p_row = src_core * (NBp * BLK) + (e_src - bounds[src_core])
        beta = p_row >> 15
        idx16 = p_row & (BUCKET - 1)

        blk = (e_dst - n0) >> 7
        dstloc = (e_dst - n0) & 127

        key = blk * 4 + beta
        order = np.argsort(key, kind="stable")
        key_s = key[order]
        cnts = np.bincount(key_s, minlength=NB * 4)
        starts = np.concatenate([[0], np.cumsum(cnts)[:-1]])
        pos = np.arange(len(key_s)) - np.repeat(starts, cnts)
        blk_s, beta_s = blk[order], beta[order]
        chunk_i = pos >> 7
        part = pos & 127
        assert (chunk_i < CB[beta_s]).all(), "bucket cap overflow"

        col = blk_s * NCH + cbase[beta_s] + chunk_i
        dst_full = np.full((NBp * NCH, BLK), -1.0, np.float32)
        idx_full = np.zeros((NBp * NCH, BLK), np.int64)
        dst_full[col, part] = dstloc[order].astype(np.float32)
        idx_full[col, part] = idx16[order]

        # device idx stream: per group, per bucket, blocks-minor
        idx_dev_cols = []
        for g in range(NGRP):
            for b in range(4):
                cols = []
                for bg in range(GBLK):
                    bb = g * GBLK + bg
                    cols.extend(bb * NCH + cbase[b] + i for i in range(CB[b]))
                seq = idx_full[cols].reshape(-1)  # [G*CB[b]*128]
                if len(seq):
                    idx_dev_cols.append(_wrap_idx(seq))
        idx_dev = np.concatenate(idx_dev_cols, axis=1)  # [128, NGRP*NCH*G*8]
        dst_dev = dst_full.T.copy()  # [128, NBp*NCH]

        def pack_nodes(vals, pad):
            pk = np.full(NBp * BLK, pad, np.float32)
            pk[:size] = vals[n0:n1]
            return pk.reshape(NBp, BLK).T.copy()  # [128, NBp]

        apk = pack_nodes(a, 0.0)
        cspk = pack_nodes(cs, 0.0)
        cdpk = pack_nodes(cd, 0.0)
        segpk = pack_nodes((seg - c * GPC).astype(np.float32), -1.0)
        invc = (1.0 / np.maximum(pre["gcounts"][c * GPC:(c + 1) * GPC], 1.0))
        invc = invc.astype(np.float32)[:, None]

        out.append(dict(idx=idx_dev, dst=dst_dev, apk=apk, cs=cspk, cd=cdpk,
                        seg=segpk, invc=invc))
    return out, dict(NB=NB, CB=tuple(int(x) for x in CB), NCH=NCH,
                     NGRP=NGRP, NBp=NBp, R=R)


def _branch_caps(pre, NB):
    """Max chunks per (block, bucket) over cores for one branch."""
    NBp = ((NB + GBLK - 1) // GBLK) * GBLK
    CB = np.zeros(4, np.int64)
    for c in range(NCORES):
        cc = pre["cores"][c]
        n0 = cc["n0"]
        e_src, e_dst = cc["e_src"], cc["e_dst"]
        src_core = np.searchsorted(pre["bounds"], e_src, side="right") - 1
        p_row = src_core * (NBp * BLK) + (e_src - pre["bounds"][src_core])
        beta = p_row >> 15
        blk = (e_dst - n0) >> 7
        cnt = np.bincount(blk * 4 + beta, minlength=NB * 4).reshape(NB, 4)
        CB = np.maximum(CB, (cnt + 127) // 128, casting="unsafe").astype(np.int64) \
            if False else np.maximum(CB, ((cnt + 127) // 128).max(axis=0))
    return CB


# --------------------------------------------------------- numpy device sim

def _run_numpy_sim(devs, meta, W):
    """Full 8-core simulation of one branch with AG between layers."""
    NBp = meta["NBp"]
    h1 = []
    W1r, b1r = W[0], W[1]
    for c in range(NCORES):
        shard = np.zeros((NBp * BLK, HIDDEN), np.float32)
        for blk in range(NBp):
            a_col = devs[c]["apk"][:, blk][:, None]
            cs_col = devs[c]["cs"][:, blk][:, None]
            t = W1r * a_col + b1r
            shard[blk * BLK:(blk + 1) * BLK] = np.maximum(t, 0.0) * cs_col
        h1.append(shard)
    tables = [np.concatenate(h1, axis=0)]
    hgs = [None] * NCORES
    cur = h1
    for li in range(3):
        table = np.concatenate(cur, axis=0)
        nxt = []
        for c in range(NCORES):
            dev = devs[c]
            s, hg = _sim_core_layer(dev, meta, W, li, table)
            if li < 2:
                nxt.append(s)
            else:
                hgs[c] = hg
        cur = nxt
    return np.concatenate(hgs, axis=0)


def _sim_core_layer(dev, meta, W, li, table):
    NB, NCH, NGRP, CB = meta["NB"], meta["NCH"], meta["NGRP"], meta["CB"]
    NBp = meta["NBp"]
    cbase = np.concatenate([[0], np.cumsum(CB)]).astype(np.int64)
    Wl = W[2 + 2 * li]
    bl = W[3 + 2 * li]
    Rr = table.shape[0]
    iota = np.arange(128, dtype=np.float32)
    newshard = np.zeros((NBp * BLK, HIDDEN), np.float32)
    psum_hg = np.zeros((16, HIDDEN), np.float32)
    colptr = 0
    idx_dev = dev["idx"]
    for g in range(NGRP):
        msg = np.zeros((128, GBLK * NCH, HIDDEN), np.float32)
        slot = 0
        for bkt in range(4):
            ncol8 = GBLK * CB[bkt] * 8
            if ncol8 == 0:
                continue
            wrapped = idx_dev[:16, colptr:colptr + ncol8]
            seq = wrapped.T.reshape(-1).astype(np.int64)
            colptr += ncol8
            view = table[bkt * BUCKET:min((bkt + 1) * BUCKET, Rr)]
            gath = view[seq]
            nslots = GBLK * CB[bkt]
            msg[:, slot:slot + nslots, :] = \
                gath.reshape(nslots, 128, HIDDEN).transpose(1, 0, 2)
            slot += nslots
        for bg in range(GBLK):
            bb = g * GBLK + bg
            agg = np.zeros((128, HIDDEN), np.float32)
            for bkt in range(4):
                for i in range(CB[bkt]):
                    s = cbase[bkt] * GBLK + bg * CB[bkt] + i
                    colx = bb * NCH + cbase[bkt] + i
                    d = dev["dst"][:, colx]
                    oh = (iota[None, :] == d[:, None]).astype(np.float32)
                    agg += oh.T @ msg[:, s, :]
            aggs = agg * dev["cd"][:, bb][:, None]
            tmpT = Wl.T @ aggs.T
            h = np.maximum(tmpT + bl[:, None], 0.0).T  # [n, f]
            if li < 2:
                newshard[bb * BLK:(bb + 1) * BLK] = \
                    h * dev["cs"][:, bb][:, None]
            else:
                segc = dev["seg"][:, bb]
                mask = (iota[None, :16] == segc[:, None]).astype(np.float32)
                psum_hg += mask.T @ h
    if li < 2:
        return newshard, None
    return None, psum_hg * dev["invc"]


# ------------------------------------------------------------- bass builder

def _build_nc(meta):
    import concourse.bacc as bacc
    import concourse.mybir as mybir
    import concourse.tile as tile

    dt = mybir.dt
    NB, CB, NCH, NGRP, NBp, R = (meta["NB"], meta["CB"], meta["NCH"],
                                 meta["NGRP"], meta["NBp"], meta["R"])
    cbase = np.concatenate([[0], np.cumsum(CB)]).astype(np.int64)
    IDXC = NCH * GBLK * 8  # idx cols per group
    nviews = [(b * BUCKET, min((b + 1) * BUCKET, R)) for b in range(4)
              if b * BUCKET < R]

    stages = os.environ.get("KBUILD_STAGES", "full")
    nlayers = {"h1": 0, "l2": 1, "l3": 2, "full": 3}[stages]
    grp_lim = int(os.environ.get("KBUILD_NGRP", "0")) or None
    nogather = bool(int(os.environ.get("KBUILD_NOGATHER", "0")))
    localtab = bool(int(os.environ.get("KBUILD_LOCALTAB", "0")))
    nc = bacc.Bacc("TRN2", target_bir_lowering=False, debug=False,
                   enable_asserts=False, num_devices=NCORES)

    ins = {}
    for br in range(2):
        ins[f"idx{br}"] = nc.dram_tensor(f"idx{br}", [128, NGRP * IDXC],
                                         dt.int16, kind="ExternalInput")
        ins[f"dst{br}"] = nc.dram_tensor(f"dst{br}", [128, NBp * NCH],
                                         dt.float32, kind="ExternalInput")
        for nm in ("apk", "cs", "cd", "seg"):
            ins[f"{nm}{br}"] = nc.dram_tensor(f"{nm}{br}", [128, NBp],
                                              dt.float32, kind="ExternalInput")
        ins[f"invc{br}"] = nc.dram_tensor(f"invc{br}", [16, 1], dt.float32,
                                          kind="ExternalInput")
    for nm in ("W1r", "b1r", "W2", "W3", "W4", "iota", "ident"):
        ins[nm] = nc.dram_tensor(nm, [128, 128], dt.float32,
                                 kind="ExternalInput")
    for nm in ("b2c", "b3c", "b4c"):
        ins[nm] = nc.dram_tensor(nm, [128, 1], dt.float32,
                                 kind="ExternalInput")
    outs = [nc.dram_tensor(f"hg{br}", [16, 128], dt.float32,
                           kind="ExternalOutput") for br in range(2)]

    with tile.TileContext(nc) as tc:
        with tc.tile_pool(name="dram", bufs=1, space="DRAM") as dram, \
             tc.tile_pool(name="const", bufs=1) as cpool, \
             tc.tile_pool(name="nodes", bufs=1) as npool, \
             tc.tile_pool(name="stream", bufs=3) as spool, \
             tc.tile_pool(name="msgp", bufs=2) as mpool, \
             tc.tile_pool(name="ohp", bufs=4) as ohpool, \
             tc.tile_pool(name="workp", bufs=3) as wpool, \
             tc.tile_pool(name="psA", bufs=2, space="PSUM") as psA, \
             tc.tile_pool(name="psB", bufs=2, space="PSUM") as psB, \
             tc.tile_pool(name="psC", bufs=2, space="PSUM") as psC, \
             tc.tile_pool(name="psD", bufs=1, space="PSUM") as psD, \
             tc.tile_pool(name="psE", bufs=1, space="PSUM") as psE:

            shard = dram.tile([NBp * BLK, 128], dt.float32)
            tables_br = [[dram.tile([R, 128], dt.float32,
                                    name=f"table{br}_{i}",
                                    tag=f"table{br}_{i}",
                                    addr_space="Shared")
                          for i in range(3)] for br in range(2)]

            consts = {}
            for nm in ("W1r", "b1r", "W2", "W3", "W4", "iota", "ident"):
                t = cpool.tile([128, 128], dt.float32, tag=nm)
                nc.sync.dma_start(t[:], ins[nm][:])
                consts[nm] = t
            for nm in ("b2c", "b3c", "b4c"):
                t = cpool.tile([128, 1], dt.float32, tag=nm)
                nc.sync.dma_start(t[:], ins[nm][:])
                consts[nm] = t

            for br in range(2):
                tables = tables_br[br]
                # per-branch node arrays resident in SBUF
                nodes = {}
                for nm in ("apk", "cs", "cd", "seg"):
                    t = npool.tile([128, NBp], dt.float32, tag=nm)
                    nc.sync.dma_start(t[:], ins[f"{nm}{br}"][:])
                    nodes[nm] = t
                invc = npool.tile([16, 1], dt.float32, tag="invc")
                nc.sync.dma_start(invc[:], ins[f"invc{br}"][:])

                # ---- h1 phase
                sid_h1, _ = nc.enter_named_scope(f"b{br}_h1", False)
                for blk in range(NBp):
                    t1 = wpool.tile([128, 128], dt.float32, tag="h1t")
                    nc.vector.scalar_tensor_tensor(
                        t1[:], consts["W1r"][:], nodes["apk"][:, blk:blk + 1],
                        consts["b1r"][:],
                        mybir.AluOpType.mult, mybir.AluOpType.add)
                    h1s = wpool.tile([128, 128], dt.float32, tag="h1s")
                    nc.vector.tensor_scalar(
                        h1s[:], t1[:], 0.0, nodes["cs"][:, blk:blk + 1],
                        mybir.AluOpType.max, mybir.AluOpType.mult)
                    nc.sync.dma_start(shard[blk * BLK:(blk + 1) * BLK, :],
                                      h1s[:])
                nc.leave_named_scope(f"b{br}_h1", sid_h1, False)
                sid_ag, _ = nc.enter_named_scope(f"b{br}_ag1", False)
                nc.gpsimd.collective_compute(
                    "AllGather", mybir.AluOpType.bypass,
                    replica_groups=[list(range(NCORES))],
                    ins=[shard.opt()], outs=[tables[0].opt()])
                nc.leave_named_scope(f"b{br}_ag1", sid_ag, False)

                # ---- layers 2..4
                for li in range(nlayers):
                    sid_l, _ = nc.enter_named_scope(f"b{br}_l{li+2}", False)
                    table = tables[li]
                    if localtab:
                        tl = dram.tile([R, 128], dt.float32, name=f"tl{br}_{li}",
                                       tag=f"tloc{li}")
                        nc.sync.dma_start(tl[:], table[:])
                        table = tl
                    Wl = consts[("W2", "W3", "W4")[li]]
                    blc = consts[("b2c", "b3c", "b4c")[li]]
                    last = (li == 2)
                    if stages != "full" and li == nlayers - 1:
                        last = False  # skip readout when truncated
                    if last:
                        hg_ps = psE.tile([16, 128], dt.float32, tag="hg")
                    for g in range(NGRP if grp_lim is None else min(grp_lim, NGRP)):
                        idxt = spool.tile([128, IDXC], dt.int16, tag="idxt")
                        nc.sync.dma_start(
                            idxt[:], ins[f"idx{br}"][:, g * IDXC:(g + 1) * IDXC])
                        dstt = spool.tile([128, GBLK * NCH], dt.float32,
                                          tag="dstt")
                        nc.sync.dma_start(
                            dstt[:],
                            ins[f"dst{br}"][:, g * GBLK * NCH:(g + 1) * GBLK * NCH])
                        msg = mpool.tile([128, GBLK * NCH, 128], dt.float32,
                                         tag="msg")
                        ccol = 0
                        for bkt in range(4):
                            L = GBLK * CB[bkt] * 128
                            if L == 0:
                                continue
                            v0, v1 = bkt * BUCKET, min((bkt + 1) * BUCKET, R)
                            s0 = cbase[bkt] * GBLK
                            nslots = GBLK * CB[bkt]
                            if nogather:
                                nc.sync.dma_start(
                                    msg[:, s0:s0 + nslots, :],
                                    table[0:nslots * 128, :].rearrange(
                                        "(p s) f -> p s f", p=128))
                            else:
                                nc.gpsimd.dma_gather(
                                    msg[:, s0:s0 + nslots, :], table[v0:v1, :],
                                    idxt[:, ccol:ccol + L // 16], L, L, 128,
                                    single_packet=False)
                            ccol += L // 16
                        for bg in range(GBLK):
                            bb = g * GBLK + bg
                            agg_ps = psA.tile([128, 128], dt.float32,
                                              tag="agg")
                            nchunk = 0
                            for bkt in range(4):
                                for i in range(CB[bkt]):
                                    s = cbase[bkt] * GBLK + bg * CB[bkt] + i
                                    colx = (bg * NCH + cbase[bkt] + i)
                                    oh = ohpool.tile([128, 128], dt.float32,
                                                     tag="oh")
                                    nc.vector.tensor_scalar(
                                        oh[:], consts["iota"][:],
                                        dstt[:, colx:colx + 1], None,
                                        mybir.AluOpType.is_equal)
                                    nc.tensor.matmul(
                                        agg_ps[:], oh[:], msg[:, s, :],
                                        start=(nchunk == 0),
                                        stop=(nchunk == NCH - 1))
                                    nchunk += 1
                            aggs = wpool.tile([128, 128], dt.float32,
                                              tag="aggs")
                            nc.vector.tensor_scalar(
                                aggs[:], agg_ps[:], nodes["cd"][:, bb:bb + 1],
                                None, mybir.AluOpType.mult)
                            aggT_ps = psB.tile([128, 128], dt.float32,
                                               tag="aggT")
                            nc.tensor.transpose(aggT_ps[:], aggs[:],
                                                consts["ident"][:])
                            aggT = wpool.tile([128, 128], dt.float32,
                                              tag="aggT_s")
                            nc.any.tensor_copy(aggT[:], aggT_ps[:])
                            tmp_ps = psC.tile([128, 128], dt.float32,
                                              tag="tmp")
                            nc.tensor.matmul(tmp_ps[:], Wl[:], aggT[:],
                                             start=True, stop=True)
                            hT = wpool.tile([128, 128], dt.float32, tag="hT")
                            nc.scalar.activation(
                                hT[:], tmp_ps[:],
                                mybir.ActivationFunctionType.Relu,
                                bias=blc[:], scale=1.0)
                            h_ps = psD.tile([128, 128], dt.float32, tag="hf")
                            nc.tensor.transpose(h_ps[:], hT[:],
                                                consts["ident"][:])
                            if not last:
                                hs = wpool.tile([128, 128], dt.float32,
                                                tag="hs")
                                nc.vector.tensor_scalar(
                                    hs[:], h_ps[:], nodes["cs"][:, bb:bb + 1],
                                    None, mybir.AluOpType.mult)
                                nc.sync.dma_start(
                                    shard[bb * BLK:(bb + 1) * BLK, :], hs[:])
                            else:
                                h4 = wpool.tile([128, 128], dt.float32,
                                                tag="hs")
                                nc.any.tensor_copy(h4[:], h_ps[:])
                                mask = ohpool.tile([128, 16], dt.float32,
                                                   tag="mask")
                                nc.vector.tensor_scalar(
                                    mask[:], consts["iota"][:, 0:16],
                                    nodes["seg"][:, bb:bb + 1], None,
                                    mybir.AluOpType.is_equal)
                                nc.tensor.matmul(
                                    hg_ps[:], mask[:], h4[:],
                                    start=(bb == 0), stop=(bb == NBp - 1))
                    nc.leave_named_scope(f"b{br}_l{li+2}", sid_l, False)
                    if not last:
                        if li + 1 < 3:
                            sid_a, _ = nc.enter_named_scope(f"b{br}_ag{li+2}", False)
                            nc.gpsimd.collective_compute(
                                "AllGather", mybir.AluOpType.bypass,
                                replica_groups=[list(range(NCORES))],
                                ins=[shard.opt()], outs=[tables[li + 1].opt()])
                            nc.leave_named_scope(f"b{br}_ag{li+2}", sid_a, False)
                    else:
                        hg_sb = wpool.tile([16, 128], dt.float32, tag="hgsb")
                        nc.vector.tensor_scalar(
                            hg_sb[:], hg_ps[:], invc[:], None,
                            mybir.AluOpType.mult)
                        nc.sync.dma_start(outs[br][:], hg_sb[:])
    nc.compile()
    return nc


# ------------------------------------------------------------------ driver

def _make_in_maps(devs1, devs2, meta, weights):
    (W1, b1, W2, b2, W3, b3, W4, b4) = weights
    W1r = np.tile(np.asarray(W1, np.float32).reshape(1, 128), (128, 1))
    b1r = np.tile(np.asarray(b1, np.float32).reshape(1, 128), (128, 1))
    iota_np = np.tile(np.arange(128, dtype=np.float32)[None, :], (128, 1))
    ident_np = np.eye(128, dtype=np.float32)
    shared = {
        "W1r": W1r, "b1r": b1r,
        "W2": np.asarray(W2, np.float32), "W3": np.asarray(W3, np.float32),
        "W4": np.asarray(W4, np.float32),
        "b2c": np.asarray(b2, np.float32)[:, None],
        "b3c": np.asarray(b3, np.float32)[:, None],
        "b4c": np.asarray(b4, np.float32)[:, None],
        "iota": iota_np, "ident": ident_np,
    }
    in_maps = []
    for c in range(NCORES):
        m = dict(shared)
        for br, devs in ((0, devs1), (1, devs2)):
            d = devs[c]
            m[f"idx{br}"] = np.ascontiguousarray(d["idx"])
            m[f"dst{br}"] = np.ascontiguousarray(d["dst"])
            m[f"apk{br}"] = np.ascontiguousarray(d["apk"])
            m[f"cs{br}"] = np.ascontiguousarray(d["cs"])
            m[f"cd{br}"] = np.ascontiguousarray(d["cd"])
            m[f"seg{br}"] = np.ascontiguousarray(d["seg"])
            m[f"invc{br}"] = d["invc"]
        in_maps.append(m)
    return in_maps


def _prepare(src1, dst1, seg1, src2, dst2, seg2):
    pre1 = _preprocess_branch(src1, dst1, seg1)
    pre2 = _preprocess_branch(src2, dst2, seg2)
    NB = int(max((np.diff(pre1["bounds"]).max() + BLK - 1) // BLK,
                 (np.diff(pre2["bounds"]).max() + BLK - 1) // BLK))
    CB = np.maximum(_branch_caps(pre1, NB), _branch_caps(pre2, NB))
    devs1, meta = _finish_packing(pre1, NB, CB)
    devs2, meta2 = _finish_packing(pre2, NB, CB)
    assert meta == meta2
    return pre1, pre2, devs1, devs2, meta


def _ensure_ntff_hook():
    """bass_utils imports antenv.axon_hooks for trace=True under axon; this
    image's antenv lacks it. Register an equivalent ctypes-based hook."""
    import sys
    try:
        import antenv.axon_hooks  # noqa: F401
        return
    except ImportError:
        pass
    import contextlib
    import ctypes
    import types

    so_path = os.environ.get("PJRT_LIBRARY_PATH", "/opt/axon/libaxon_pjrt.so")

    def _make_hook():
        try:
            lib = ctypes.CDLL(so_path)
        except OSError:
            return None
        if not hasattr(lib, "axon_start_nrt_profile"):
            return None
        lib.axon_start_nrt_profile.argtypes = [
            ctypes.POINTER(ctypes.c_int64), ctypes.c_size_t]
        lib.axon_start_nrt_profile.restype = ctypes.c_int64
        lib.axon_stop_nrt_profile.argtypes = [ctypes.c_char_p]
        lib.axon_stop_nrt_profile.restype = ctypes.c_int64

        @contextlib.contextmanager
        def _hook(output_dir, device_ids):
            import jax
            jax.devices()
            if device_ids:
                ids = (ctypes.c_int64 * len(device_ids))(*device_ids)
                rc = lib.axon_start_nrt_profile(ids, len(device_ids))
            else:
                rc = lib.axon_start_nrt_profile(None, 0)
            if rc != 0:
                raise RuntimeError(f"axon_start_nrt_profile rc={rc}")
            try:
                yield
            finally:
                n = lib.axon_stop_nrt_profile(str(output_dir).encode())
                if n <= 0:
                    print(f"ntff profile: {n} files written", flush=True)

        return _hook

    mod = types.ModuleType("antenv.axon_hooks")
    _the_hook = _make_hook()
    mod.get_axon_ntff_profile_hook = lambda: _the_hook
    mod.set_axon_ntff_profile_hook = lambda h: None
    sys.modules["antenv.axon_hooks"] = mod


def kernel(src1, dst1, seg1, src2, dst2, seg2,
           W1, b1, W2, b2, W3, b3, W4, b4, Wc, bc):
    global LAST_RESULTS
    from concourse.bass_utils import run_bass_kernel_spmd

    pre1, pre2, devs1, devs2, meta = _prepare(src1, dst1, seg1,
                                              src2, dst2, seg2)
    weights = (W1, b1, W2, b2, W3, b3, W4, b4)
    in_maps = _make_in_maps(devs1, devs2, meta,
                            tuple(np.asarray(w, np.float32) for w in weights))

    key = (meta["NB"], meta["CB"], os.environ.get("KBUILD_STAGES", "full"),
           os.environ.get("KBUILD_NGRP", "0"),
           os.environ.get("KBUILD_NOGATHER", "0"),
           os.environ.get("KBUILD_LOCALTAB", "0"))
    if key not in _NC_CACHE:
        _NC_CACHE[key] = _build_nc(meta)
    nc = _NC_CACHE[key]

    trace = bool(int(os.environ.get("KERNEL_TRACE", "0")))
    if trace:
        _ensure_ntff_hook()
    res = run_bass_kernel_spmd(nc, in_maps, core_ids=list(range(NCORES)),
                               trace=trace)
    LAST_RESULTS = res
    hg1 = np.concatenate([res.results[c]["hg0"] for c in range(NCORES)], 0)
    hg2 = np.concatenate([res.results[c]["hg1"] for c in range(NCORES)], 0)
    hg = np.abs(hg1 - hg2)
    logits = hg @ np.asarray(Wc, np.float32) + np.asarray(bc, np.float32)
    return (hg1.astype(np.float32), hg2.astype(np.float32),
            logits.astype(np.float32))


def kernel_numpy(src1, dst1, seg1, src2, dst2, seg2,
                 W1, b1, W2, b2, W3, b3, W4, b4, Wc, bc):
    """Host-only simulation of the exact device algorithm (for validation)."""
    pre1, pre2, devs1, devs2, meta = _prepare(src1, dst1, seg1,
                                              src2, dst2, seg2)
    W1r = np.tile(np.asarray(W1, np.float32).reshape(1, 128), (128, 1))
    b1r = np.tile(np.asarray(b1, np.float32).reshape(1, 128), (128, 1))
    W = (W1r, b1r, np.asarray(W2, np.float32), np.asarray(b2, np.float32),
         np.asarray(W3, np.float32), np.asarray(b3, np.float32),
         np.asarray(W4, np.float32), np.asarray(b4, np.float32))
    hg1 = _run_numpy_sim(devs1, meta, W)
    hg2 = _run_numpy_sim(devs2, meta, W)
    hg = np.abs(hg1 - hg2)
    logits = hg @ np.asarray(Wc, np.float32) + np.asarray(bc, np.float32)
    return hg1, hg2, logits
